# revision 12
# baseline (speedup 1.0000x reference)
"""Trainium2 Bass kernel for the pickup/delivery heterogeneous MHA module.

Shapes (hardcoded): q (16, 501, 128) f32, 8 heads, key dim 16,
n_pick = n_delivery = 250, G = 1 + 250 + 250 = 501.

Sharding: data parallel over batch — 2 batches per core on 8 cores.
"""

import sys

for _p in ("/opt/trn_rl_repo", "/root/.axon_site/_ro/trn_rl_repo"):
    if _p not in sys.path:
        sys.path.insert(0, _p)

import math

import ml_dtypes
import numpy as np

B, G, D, H, KD = 16, 501, 128, 8, 16
NP = ND = 250
NCORES = 8
BPC = B // NCORES  # batches per core
F32 = None  # set after imports
BF16 = None

# rotated g order: [picks (g 1..251), delivs (g 251..501), depot (g 0)]
ROT = np.concatenate([np.arange(1, G), [0]])

# main g-chunks in rotated coords (partition-tiles of the key/value axis)
CHUNKS_MAIN = [(0, 128), (128, 250), (250, 378), (378, 501)]
# score slot stream: 4 main chunks + 2 pick-block + 2 deliv-block chunks
# (kind, c0, c1): kind m = vs full QT (N=501), p/d = vs QP/QD (N=500)
SLOTS = [
    ("m", 0, 128), ("m", 128, 250), ("m", 250, 378), ("m", 378, 501),
    ("p", 0, 128), ("p", 128, 250), ("d", 250, 378), ("d", 378, 500),
]
# head emission order: even heads hit row/col groups 0..3, then odd heads
HEAD_ORDER = [0, 2, 4, 6, 1, 3, 5, 7]
# vext chunk index holding rows [c0, c1) (rotated)
VCHUNK = {0: 0, 128: 1, 250: 2, 378: 3}

_CACHE = {}


def _build_nc(loop_k=0):
    """loop_k=0: normal kernel. loop_k>0: wrap the body in a device-side
    For_i loop of loop_k iterations (benchmarking only)."""
    import contextlib

    import concourse.bacc as bacc
    import concourse.mybir as mybir
    import concourse.tile as tile

    f32 = mybir.dt.float32
    bf16 = mybir.dt.bfloat16
    EXP = mybir.ActivationFunctionType.Exp

    nc = bacc.Bacc("TRN2", target_bir_lowering=False, debug=False,
                   num_devices=NCORES)

    hq = nc.dram_tensor("hq", [BPC, D, G], f32, kind="ExternalInput")
    wnames = ["wq", "wke", "wko", "w1", "w2", "w3", "w4", "wv"]
    wdr = {n: nc.dram_tensor(n, [D, D], f32, kind="ExternalInput")
           for n in wnames}
    wo0 = nc.dram_tensor("wo0", [D, D], bf16, kind="ExternalInput")
    wo1 = nc.dram_tensor("wo1", [D, D], bf16, kind="ExternalInput")
    seldr = nc.dram_tensor("sel", [D, D], f32, kind="ExternalInput")
    out = nc.dram_tensor("out", [BPC, G, D], f32, kind="ExternalOutput")

    with tile.TileContext(nc) as tc:
        with (
            tc.tile_pool(name="const", bufs=1) as constp,
            tc.tile_pool(name="perb", bufs=2) as perb,
            tc.tile_pool(name="vext", bufs=2) as vextp,
            tc.tile_pool(name="expp", bufs=6) as expp,
            tc.tile_pool(name="tmp", bufs=3) as tmpp,
            tc.tile_pool(name="outp", bufs=3) as outp,
            tc.tile_pool(name="ps", bufs=3, space="PSUM") as psp,
            tc.tile_pool(name="hps", bufs=2, space="PSUM") as hpsp,
        ):
            wsb = {}
            for n in wnames:
                t = constp.tile([D, D], f32, name=f"{n}_sb")
                nc.sync.dma_start(t[:], wdr[n].ap())
                wsb[n] = t
            wo0_sb = constp.tile([D, D], bf16, name="wo0_sb")
            nc.sync.dma_start(wo0_sb[:], wo0.ap())
            wo1_sb = constp.tile([D, D], bf16, name="wo1_sb")
            nc.sync.dma_start(wo1_sb[:], wo1.ap())
            sel_sb = constp.tile([D, D], f32, name="sel_sb")
            nc.sync.dma_start(sel_sb[:], seldr.ap())

            # warm the ACT exp table so its one-time load overlaps the
            # initial weight DMAs instead of stalling the first real exp
            wtile = constp.tile([1, 4], f32, name="wtile")
            nc.gpsimd.memset(wtile[:], 0.0)
            wtile2 = constp.tile([1, 4], f32, name="wtile2")
            nc.scalar.activation(wtile2[:], wtile[:], EXP)

            def proj_dma(b, st):
                hT = perb.tile([D, G], f32, name="hT")
                nc.sync.dma_start(hT[:], hq.ap()[b])
                st["hT"] = hT

            def proj_a(b, st):
                """QT | KTe (fp32, all heads stacked: M = 128)."""
                hT = st["hT"]
                pa = psp.tile([128, 2, 512], f32, tag="sc", name="pa")
                nc.tensor.matmul(pa[:, 0, 0:G], wsb["wq"][:], hT[:])
                nc.tensor.matmul(pa[:, 1, 0:G], wsb["wke"][:], hT[:])
                qt = perb.tile([D, G], bf16, name="qt")
                nc.vector.tensor_copy(qt[:], pa[:, 0, 0:G])
                kte = perb.tile([D, G], bf16, name="kte")
                nc.vector.tensor_copy(kte[:], pa[:, 1, 0:G])
                st["qt"], st["kte"] = qt, kte

            def proj_b(b, st):
                """KTo | QP."""
                hT = st["hT"]
                pb = psp.tile([128, 2, 512], f32, tag="sc", name="pb")
                nc.tensor.matmul(pb[:, 0, 0:G], wsb["wko"][:], hT[:])
                nc.tensor.matmul(pb[:, 1, 0:250], wsb["w1"][:],
                                 hT[:, 0:250], start=True, stop=False)
                nc.tensor.matmul(pb[:, 1, 250:500], wsb["w3"][:],
                                 hT[:, 250:500], start=False, stop=True,
                                 skip_group_check=True)
                kto = perb.tile([D, G], bf16, name="kto")
                nc.vector.tensor_copy(kto[:], pb[:, 0, 0:G])
                qp = perb.tile([D, 500], bf16, name="qp")
                nc.vector.tensor_copy(qp[:], pb[:, 1, 0:500])
                st["kto"], st["qp"] = kto, qp

            def proj_c(b, st):
                """QD | V-chunks. V_ext tiles hold per chunk
                [128, 8 heads * (16 V + 16 ones)]; the ones columns make the
                attn@V matmuls accumulate softmax denominators for free."""
                hT = st["hT"]
                pc = psp.tile([128, 2, 512], f32, tag="sc", name="pc")
                nc.tensor.matmul(pc[:, 0, 0:250], wsb["w2"][:],
                                 hT[:, 0:250], start=True, stop=False)
                nc.tensor.matmul(pc[:, 0, 250:500], wsb["w4"][:],
                                 hT[:, 250:500], start=False, stop=True,
                                 skip_group_check=True)
                for ci, (c0, c1) in enumerate(CHUNKS_MAIN):
                    nc.tensor.matmul(
                        pc[0:c1 - c0, 1, 128 * ci:128 * ci + 128],
                        hT[:, c0:c1], wsb["wv"][:],
                        start=(ci == 0), stop=(ci == 3),
                        skip_group_check=True)
                qd = perb.tile([D, 500], bf16, name="qd")
                nc.vector.tensor_copy(qd[:], pc[:, 0, 0:500])
                vext = []
                for ci, (c0, c1) in enumerate(CHUNKS_MAIN):
                    cs = c1 - c0
                    vt = vextp.tile([128, 256], bf16, tag=f"v{ci}",
                                    name=f"vext{ci}")
                    vv = vt.rearrange("p (h w) -> p h w", h=H)
                    nc.gpsimd.memset(vv[:, :, 16:32], 1.0)
                    src = pc[0:cs, 1, 128 * ci:128 * ci + 128]
                    nc.vector.tensor_copy(
                        vv[0:cs, :, 0:16],
                        src.rearrange("p (h v) -> p h v", h=H))
                    vext.append(vt)
                st["qd"], st["vext"] = qd, vext

            def stream_units(b, st):
                """32 units: each = 2 score matmuls -> exp -> 2 attn@V."""
                H0 = hpsp.tile([128, 512], f32, tag="H", name="H0")
                H1 = hpsp.tile([128, 512], f32, tag="H", name="H1")
                st["Hs"] = (H0, H1)
                nslots = len(SLOTS) * H  # 64

                def unit(u):
                    def emit():
                        cur = psp.tile([128, 2, 512], f32, tag="sc",
                                       name="sc")
                        pend = []
                        for sub in range(2):
                            t = 2 * u + sub
                            s, j = t // H, t % H
                            h = HEAD_ORDER[j]
                            kind, c0, c1 = SLOTS[s]
                            cs = c1 - c0
                            p, par = h // 2, h % 2
                            kt = st["kte"] if par == 0 else st["kto"]
                            rhs = {"m": st["qt"], "p": st["qp"],
                                   "d": st["qd"]}[kind]
                            rhs = rhs[32 * p:32 * p + 32, :]
                            nc.tensor.matmul(
                                cur[0:cs, sub, 0:rhs.shape[1]],
                                kt[32 * p:32 * p + 32, c0:c1], rhs,
                                tile_position=(32 * p, 0))
                            pend.append((sub, s, h))
                        ex = expp.tile([128, 2, 501], bf16, name="ex")
                        nc.scalar.activation(ex[:], cur[:, :, 0:G], EXP)
                        for sub, s, h in pend:
                            kind, c0, c1 = SLOTS[s]
                            cs = c1 - c0
                            n = G if kind == "m" else 500
                            vt = st["vext"][VCHUNK[c0]]
                            quad, cg = h // 4, 32 * (h % 4)
                            Hq = H0 if quad == 0 else H1
                            # PSUM pending-zero is per partition range:
                            # each head's col-group needs its own start
                            nc.tensor.matmul(
                                Hq[cg:cg + 32, 0:n],
                                vt[0:cs, 32 * h:32 * h + 32],
                                ex[0:cs, sub, 0:n],
                                start=(s == 0), stop=(s == len(SLOTS) - 1),
                                tile_position=(0, cg),
                                skip_group_check=True)
                    return emit

                return [unit(u) for u in range(nslots // 2)]

            def tail_norm(b, st):
                """normalize by softmax denominators. The sel-matmul
                broadcasts each 32-group's denominator row to the whole
                group (engine partition accesses must be 32-aligned, so row
                32j+16 cannot be sliced directly)."""
                hns = []
                for quad, Hq in ((0, st["Hs"][0]), (1, st["Hs"][1])):
                    hsb = tmpp.tile([D, G], f32, tag="hsb", name="hsb")
                    nc.vector.tensor_copy(hsb[:], Hq[:, 0:G])
                    denb = psp.tile([128, 2, 512], f32, tag="sc",
                                    name="denb")
                    nc.tensor.matmul(denb[:, 0, 0:G], sel_sb[:], hsb[:])
                    rcb = tmpp.tile([D, G], f32, tag="rcb", name="rcb")
                    nc.vector.reciprocal_approx_fast(rcb[:],
                                                     denb[:, 0, 0:G])
                    hn = perb.tile([D, G], bf16, name=f"hn{quad}")
                    nc.vector.tensor_mul(hn[:], hsb[:], rcb[:])
                    hns.append(hn)
                st["hns"] = hns

            def tail_final(b, st):
                """out = sum_h headsT_h @ W_out_h; contraction over all 128
                rows per quad (W_out rows at denominator positions = 0)."""
                hns = st["hns"]
                for ci, (c0, c1) in enumerate(CHUNKS_MAIN):
                    cs = c1 - c0
                    ops = psp.tile([128, 2, 512], f32, tag="sc", name="ops")
                    nc.tensor.matmul(ops[0:cs, 0, 0:D], hns[0][:, c0:c1],
                                     wo0_sb[:], start=True, stop=False)
                    nc.tensor.matmul(ops[0:cs, 0, 0:D], hns[1][:, c0:c1],
                                     wo1_sb[:], start=False, stop=True,
                                     skip_group_check=True)
                    osb = outp.tile([128, D], f32, name="osb")
                    nc.vector.tensor_copy(osb[0:cs, :], ops[0:cs, 0, 0:D])
                    if ci < 3:
                        nc.sync.dma_start(out.ap()[b, c0 + 1:c1 + 1, :],
                                          osb[0:cs, :])
                    else:
                        nc.sync.dma_start(out.ap()[b, c0 + 1:G, :],
                                          osb[0:cs - 1, :])
                        nc.sync.dma_start(out.ap()[b, 0:1, :],
                                          osb[cs - 1:cs, :])

            loop_cm = (tc.For_i(0, loop_k, 1) if loop_k
                       else contextlib.nullcontext())
            with loop_cm:
                # software-pipeline the two batches: b1's projections and
                # b0's tail are interleaved into the stream emission so the
                # exp engine (the bottleneck) never runs dry at boundaries
                st0, st1 = {}, {}
                proj_dma(0, st0)
                proj_a(0, st0)
                proj_b(0, st0)
                proj_c(0, st0)
                u0 = stream_units(0, st0)
                for f in u0[0:8]:
                    f()
                proj_dma(1, st1)
                proj_a(1, st1)
                for f in u0[8:14]:
                    f()
                proj_b(1, st1)
                for f in u0[14:20]:
                    f()
                proj_c(1, st1)
                for f in u0[20:32]:
                    f()
                tail_norm(0, st0)
                u1 = stream_units(1, st1)
                for f in u1[0:8]:
                    f()
                tail_final(0, st0)
                for f in u1[8:32]:
                    f()
                tail_norm(1, st1)
                tail_final(1, st1)

    nc.compile()
    return nc


def _prep_weights(W_query, W_key, W_val, W1, W2, W3, W4, W_out):
    nf = 0.25  # 1/sqrt(16), exact power of two
    stack = lambda w: np.ascontiguousarray(
        np.asarray(w, np.float32).transpose(1, 0, 2).reshape(D, D))
    wq = stack(W_query) * nf
    wk = stack(W_key)
    mask = np.zeros((1, D), np.float32)
    for h in range(H):
        if h % 2 == 0:
            mask[0, h * KD:(h + 1) * KD] = 1.0
    wke = wk * mask
    wko = wk * (1.0 - mask)
    wo = np.asarray(W_out, np.float32)
    # W_out rows interleaved into 32-row groups: rows 32j+v hold head
    # (quad*4+j) vector v, rows 32j+16.. (denominator rows) are zero
    wo_pad = np.zeros((2, D, D), np.float32)
    for quad in range(2):
        for j in range(4):
            wo_pad[quad, 32 * j:32 * j + KD] = wo[quad * 4 + j]
    # sel[p, p'] = 1 iff p is the denominator row of p's 32-group
    sel = np.zeros((D, D), np.float32)
    for p2 in range(D):
        sel[32 * (p2 // 32) + 16, p2] = 1.0
    return {
        "wq": wq, "wke": wke, "wko": wko,
        "w1": stack(W1) * nf, "w2": stack(W2) * nf,
        "w3": stack(W3) * nf, "w4": stack(W4) * nf,
        "wv": stack(W_val), "sel": sel,
        "wo0": wo_pad[0].astype(ml_dtypes.bfloat16),
        "wo1": wo_pad[1].astype(ml_dtypes.bfloat16),
    }


def _numpy_fallback(q, W_query, W_key, W_val, W1, W2, W3, W4, W_out,
                    n_pick, n_delivery):
    """Pure-numpy reference for unexpected n_pick/n_delivery (not used for
    the standard 250/250 problem)."""
    h = np.asarray(q, np.float64)
    Bq, Gq, _ = h.shape
    nf = 1.0 / math.sqrt(KD)
    NEG = -np.inf
    proj = lambda x, W: np.einsum("bnd,hdk->hbnk", x, np.asarray(W, np.float64))
    sc = lambda Q, K: nf * np.einsum("hbqk,hbgk->hbqg", Q, K)
    zm = lambda c: np.where(c == 0, NEG, c)
    Q, K, V = proj(h, W_query), proj(h, W_key), proj(h, W_val)
    comp = sc(Q, K)
    hp, hd = h[:, 1:1 + n_pick], h[:, 1 + n_pick:]
    Kp, Vp = proj(hp, W_key), proj(hp, W_val)
    Kd, Vd = proj(hd, W_key), proj(hd, W_val)
    c_pp = zm(sc(proj(hp, W1), Kp))
    c_pd = zm(sc(proj(hp, W2), Kd))
    c_dp = zm(sc(proj(hd, W3), Kp))
    c_dd = zm(sc(proj(hd, W4), Kd))

    def place(blk, r0):
        full = np.full((H, Bq, Gq, blk.shape[3]), NEG)
        full[:, :, r0:r0 + blk.shape[2], :] = blk
        return full

    md = hd.shape[1]
    cf = np.concatenate([comp, place(c_pp, 1), place(c_pd, 1),
                         place(c_dd, Gq - md), place(c_dp, Gq - md)], axis=-1)
    cf -= cf.max(axis=-1, keepdims=True)
    e = np.exp(cf)
    attn = e / e.sum(axis=-1, keepdims=True)
    g, mp = Gq, n_pick
    heads = np.einsum("hbqg,hbgv->hbqv", attn[..., :g], V)
    heads += np.einsum("hbqp,hbpv->hbqv", attn[..., g:g + mp], Vp)
    heads += np.einsum("hbqd,hbdv->hbqv", attn[..., g + mp:g + mp + md], Vd)
    heads += np.einsum("hbqd,hbdv->hbqv",
                       attn[..., g + mp + md:g + mp + 2 * md], Vd)
    heads += np.einsum("hbqp,hbpv->hbqv", attn[..., g + mp + 2 * md:], Vp)
    return np.einsum("hbqv,hve->bqe", heads,
                     np.asarray(W_out, np.float64)).astype(np.float32)


def kernel(q, W_query, W_key, W_val, W1_query, W2_query, W3_query, W4_query,
           W_out, n_pick, n_delivery):
    np_, nd_ = int(n_pick), int(n_delivery)
    q = np.asarray(q, np.float32)
    if np_ != NP or nd_ != ND or q.shape != (B, G, D):
        return _numpy_fallback(q, W_query, W_key, W_val, W1_query, W2_query,
                               W3_query, W4_query, W_out, np_, nd_)

    from concourse import bass_utils

    if "nc" not in _CACHE:
        _CACHE["nc"] = _build_nc()
    nc = _CACHE["nc"]

    w = _prep_weights(W_query, W_key, W_val, W1_query, W2_query, W3_query,
                      W4_query, W_out)
    # host layout: rotate g axis (picks, delivs, depot) and transpose to
    # [b, d, g] so each core DMAs contiguous [128, 501] tiles
    hTr = np.ascontiguousarray(q[:, ROT, :].transpose(0, 2, 1))

    in_maps = [dict(w, hq=hTr[BPC * c:BPC * (c + 1)]) for c in range(NCORES)]
    res = bass_utils.run_bass_kernel_spmd(nc, in_maps,
                                          core_ids=list(range(NCORES)))
    return np.concatenate([r["out"] for r in res.results], axis=0)


# revision 14
# speedup vs baseline: 1.0432x; 1.0432x over previous
"""Trainium2 Bass kernel for the pickup/delivery heterogeneous MHA module.

Shapes (hardcoded): q (16, 501, 128) f32, 8 heads, key dim 16,
n_pick = n_delivery = 250, G = 1 + 250 + 250 = 501.

Sharding: data parallel over batch — 2 batches per core on 8 cores.
"""

import sys

for _p in ("/opt/trn_rl_repo", "/root/.axon_site/_ro/trn_rl_repo"):
    if _p not in sys.path:
        sys.path.insert(0, _p)

import math

import ml_dtypes
import numpy as np

B, G, D, H, KD = 16, 501, 128, 8, 16
NP = ND = 250
NCORES = 8
BPC = B // NCORES  # batches per core
F32 = None  # set after imports
BF16 = None

# rotated g order: [picks (g 1..251), delivs (g 251..501), depot (g 0)]
ROT = np.concatenate([np.arange(1, G), [0]])

# main g-chunks in rotated coords (partition-tiles of the key/value axis)
CHUNKS_MAIN = [(0, 128), (128, 250), (250, 378), (378, 501)]
# score slot stream: 4 main chunks + 2 pick-block + 2 deliv-block chunks
# (kind, c0, c1): kind m = vs full QT (N=501), p/d = vs QP/QD (N=500)
SLOTS = [
    ("m", 0, 128), ("m", 128, 250), ("m", 250, 378), ("m", 378, 501),
    ("p", 0, 128), ("p", 128, 250), ("d", 250, 378), ("d", 378, 500),
]
# head emission order: even heads hit row/col groups 0..3, then odd heads
HEAD_ORDER = [0, 2, 4, 6, 1, 3, 5, 7]
# vext chunk index holding rows [c0, c1) (rotated)
VCHUNK = {0: 0, 128: 1, 250: 2, 378: 3}

_CACHE = {}


def _build_nc(loop_k=0):
    """loop_k=0: normal kernel. loop_k>0: wrap the body in a device-side
    For_i loop of loop_k iterations (benchmarking only)."""
    import contextlib

    import concourse.bacc as bacc
    import concourse.mybir as mybir
    import concourse.tile as tile

    f32 = mybir.dt.float32
    bf16 = mybir.dt.bfloat16
    EXP = mybir.ActivationFunctionType.Exp

    nc = bacc.Bacc("TRN2", target_bir_lowering=False, debug=False,
                   num_devices=NCORES)

    hq = nc.dram_tensor("hq", [BPC, D, G], f32, kind="ExternalInput")
    wnames = ["wq", "wke", "wko", "w1", "w2", "w3", "w4", "wv"]
    wdr = {n: nc.dram_tensor(n, [D, D], f32, kind="ExternalInput")
           for n in wnames}
    wo0 = nc.dram_tensor("wo0", [D, D], bf16, kind="ExternalInput")
    wo1 = nc.dram_tensor("wo1", [D, D], bf16, kind="ExternalInput")
    seldr = nc.dram_tensor("sel", [D, D], f32, kind="ExternalInput")
    out = nc.dram_tensor("out", [BPC, G, D], f32, kind="ExternalOutput")

    with tile.TileContext(nc) as tc:
        with (
            tc.tile_pool(name="const", bufs=1) as constp,
            tc.tile_pool(name="perb", bufs=2) as perb,
            tc.tile_pool(name="vext", bufs=2) as vextp,
            tc.tile_pool(name="expp", bufs=6) as expp,
            tc.tile_pool(name="tmp", bufs=3) as tmpp,
            tc.tile_pool(name="outp", bufs=3) as outp,
            tc.tile_pool(name="ps", bufs=3, space="PSUM") as psp,
            tc.tile_pool(name="hps", bufs=2, space="PSUM") as hpsp,
        ):
            wsb = {}
            for n in wnames:
                t = constp.tile([D, D], f32, name=f"{n}_sb")
                nc.sync.dma_start(t[:], wdr[n].ap())
                wsb[n] = t
            wo0_sb = constp.tile([D, D], bf16, name="wo0_sb")
            nc.sync.dma_start(wo0_sb[:], wo0.ap())
            wo1_sb = constp.tile([D, D], bf16, name="wo1_sb")
            nc.sync.dma_start(wo1_sb[:], wo1.ap())
            sel_sb = constp.tile([D, D], f32, name="sel_sb")
            nc.sync.dma_start(sel_sb[:], seldr.ap())

            # warm the ACT exp table so its one-time load overlaps the
            # initial weight DMAs instead of stalling the first real exp
            wtile = constp.tile([1, 4], f32, name="wtile")
            nc.gpsimd.memset(wtile[:], 0.0)
            wtile2 = constp.tile([1, 4], f32, name="wtile2")
            nc.scalar.activation(wtile2[:], wtile[:], EXP)

            def proj_dma(b, st):
                hT = perb.tile([D, G], f32, name="hT")
                nc.sync.dma_start(hT[:], hq.ap()[b])
                st["hT"] = hT

            def proj_a(b, st):
                """QT | KTe (fp32, all heads stacked: M = 128)."""
                hT = st["hT"]
                pa = psp.tile([128, 2, 512], f32, tag="sc", name="pa")
                nc.tensor.matmul(pa[:, 0, 0:G], wsb["wq"][:], hT[:])
                nc.tensor.matmul(pa[:, 1, 0:G], wsb["wke"][:], hT[:])
                qt = perb.tile([D, G], bf16, name="qt")
                nc.vector.tensor_copy(qt[:], pa[:, 0, 0:G])
                kte = perb.tile([D, G], bf16, name="kte")
                nc.vector.tensor_copy(kte[:], pa[:, 1, 0:G])
                st["qt"], st["kte"] = qt, kte

            def proj_b(b, st):
                """KTo | QP."""
                hT = st["hT"]
                pb = psp.tile([128, 2, 512], f32, tag="sc", name="pb")
                nc.tensor.matmul(pb[:, 0, 0:G], wsb["wko"][:], hT[:])
                nc.tensor.matmul(pb[:, 1, 0:250], wsb["w1"][:],
                                 hT[:, 0:250], start=True, stop=False)
                nc.tensor.matmul(pb[:, 1, 250:500], wsb["w3"][:],
                                 hT[:, 250:500], start=False, stop=True,
                                 skip_group_check=True)
                kto = perb.tile([D, G], bf16, name="kto")
                nc.vector.tensor_copy(kto[:], pb[:, 0, 0:G])
                qp = perb.tile([D, 500], bf16, name="qp")
                nc.vector.tensor_copy(qp[:], pb[:, 1, 0:500])
                st["kto"], st["qp"] = kto, qp

            def proj_c(b, st):
                """QD | V-chunks. V_ext tiles hold per chunk
                [128, 8 heads * (16 V + 16 ones)]; the ones columns make the
                attn@V matmuls accumulate softmax denominators for free."""
                hT = st["hT"]
                pc = psp.tile([128, 2, 512], f32, tag="sc", name="pc")
                nc.tensor.matmul(pc[:, 0, 0:250], wsb["w2"][:],
                                 hT[:, 0:250], start=True, stop=False)
                nc.tensor.matmul(pc[:, 0, 250:500], wsb["w4"][:],
                                 hT[:, 250:500], start=False, stop=True,
                                 skip_group_check=True)
                for ci, (c0, c1) in enumerate(CHUNKS_MAIN):
                    nc.tensor.matmul(
                        pc[0:c1 - c0, 1, 128 * ci:128 * ci + 128],
                        hT[:, c0:c1], wsb["wv"][:],
                        start=(ci == 0), stop=(ci == 3),
                        skip_group_check=True)
                qd = perb.tile([D, 500], bf16, name="qd")
                nc.vector.tensor_copy(qd[:], pc[:, 0, 0:500])
                vext = []
                for ci, (c0, c1) in enumerate(CHUNKS_MAIN):
                    cs = c1 - c0
                    vt = vextp.tile([128, 256], bf16, tag=f"v{ci}",
                                    name=f"vext{ci}")
                    vv = vt.rearrange("p (h w) -> p h w", h=H)
                    nc.gpsimd.memset(vv[:, :, 16:32], 1.0)
                    src = pc[0:cs, 1, 128 * ci:128 * ci + 128]
                    nc.vector.tensor_copy(
                        vv[0:cs, :, 0:16],
                        src.rearrange("p (h v) -> p h v", h=H))
                    vext.append(vt)
                st["qd"], st["vext"] = qd, vext

            def stream_units(b, st):
                """32 units: each = 2 score matmuls -> exp -> 2 attn@V."""
                H0 = hpsp.tile([128, 512], f32, tag="H", name="H0")
                H1 = hpsp.tile([128, 512], f32, tag="H", name="H1")
                st["Hs"] = (H0, H1)
                nslots = len(SLOTS) * H  # 64

                def unit(u):
                    pend = []

                    def emit_scores():
                        """2 score matmuls into a psum tile + the exp."""
                        cur = psp.tile([128, 2, 512], f32, tag="sc",
                                       name="sc")
                        for sub in range(2):
                            t = 2 * u + sub
                            s, j = t // H, t % H
                            h = HEAD_ORDER[j]
                            kind, c0, c1 = SLOTS[s]
                            cs = c1 - c0
                            p, par = h // 2, h % 2
                            kt = st["kte"] if par == 0 else st["kto"]
                            rhs = {"m": st["qt"], "p": st["qp"],
                                   "d": st["qd"]}[kind]
                            rhs = rhs[32 * p:32 * p + 32, :]
                            nc.tensor.matmul(
                                cur[0:cs, sub, 0:rhs.shape[1]],
                                kt[32 * p:32 * p + 32, c0:c1], rhs,
                                tile_position=(32 * p, 0))
                            pend.append((sub, s, h))
                        ex = expp.tile([128, 2, 501], bf16, name="ex")
                        nc.scalar.activation(ex[:], cur[:, :, 0:G], EXP)
                        pend.append(ex)

                    def emit_avs():
                        ex = pend.pop()
                        for sub, s, h in pend:
                            kind, c0, c1 = SLOTS[s]
                            cs = c1 - c0
                            n = G if kind == "m" else 500
                            vt = st["vext"][VCHUNK[c0]]
                            quad, cg = h // 4, 32 * (h % 4)
                            Hq = H0 if quad == 0 else H1
                            # PSUM pending-zero is per partition range:
                            # each head's col-group needs its own start
                            nc.tensor.matmul(
                                Hq[cg:cg + 32, 0:n],
                                vt[0:cs, 32 * h:32 * h + 32],
                                ex[0:cs, sub, 0:n],
                                start=(s == 0), stop=(s == len(SLOTS) - 1),
                                tile_position=(0, cg),
                                skip_group_check=True)
                    return emit_scores, emit_avs

                return [unit(u) for u in range(nslots // 2)]

            def tail_norm(b, st):
                """normalize by softmax denominators. The sel-matmul
                broadcasts each 32-group's denominator row to the whole
                group (engine partition accesses must be 32-aligned, so row
                32j+16 cannot be sliced directly)."""
                hns = []
                for quad, Hq in ((0, st["Hs"][0]), (1, st["Hs"][1])):
                    hsb = tmpp.tile([D, G], f32, tag="hsb", name="hsb")
                    nc.vector.tensor_copy(hsb[:], Hq[:, 0:G])
                    denb = psp.tile([128, 2, 512], f32, tag="sc",
                                    name="denb")
                    nc.tensor.matmul(denb[:, 0, 0:G], sel_sb[:], hsb[:])
                    rcb = tmpp.tile([D, G], f32, tag="rcb", name="rcb")
                    nc.vector.reciprocal_approx_fast(rcb[:],
                                                     denb[:, 0, 0:G])
                    hn = perb.tile([D, G], bf16, name=f"hn{quad}")
                    nc.vector.tensor_mul(hn[:], hsb[:], rcb[:])
                    hns.append(hn)
                st["hns"] = hns

            def tail_final(b, st):
                """out = sum_h headsT_h @ W_out_h; contraction over all 128
                rows per quad (W_out rows at denominator positions = 0)."""
                hns = st["hns"]
                for ci, (c0, c1) in enumerate(CHUNKS_MAIN):
                    cs = c1 - c0
                    ops = psp.tile([128, 2, 512], f32, tag="sc", name="ops")
                    nc.tensor.matmul(ops[0:cs, 0, 0:D], hns[0][:, c0:c1],
                                     wo0_sb[:], start=True, stop=False)
                    nc.tensor.matmul(ops[0:cs, 0, 0:D], hns[1][:, c0:c1],
                                     wo1_sb[:], start=False, stop=True,
                                     skip_group_check=True)
                    osb = outp.tile([128, D], f32, name="osb")
                    nc.vector.tensor_copy(osb[0:cs, :], ops[0:cs, 0, 0:D])
                    if ci < 3:
                        nc.sync.dma_start(out.ap()[b, c0 + 1:c1 + 1, :],
                                          osb[0:cs, :])
                    else:
                        nc.sync.dma_start(out.ap()[b, c0 + 1:G, :],
                                          osb[0:cs - 1, :])
                        nc.sync.dma_start(out.ap()[b, 0:1, :],
                                          osb[cs - 1:cs, :])

            def emit_stream(units, aux):
                """Software-pipelined emission: unit u's attn@V matmuls are
                emitted after unit u+1's scores, so PE never waits on the
                exp it just requested. aux closures are injected at unit
                boundaries."""
                prev_av = None
                for i, (sc, av) in enumerate(units):
                    for f in aux.get(i, ()):
                        f()
                    sc()
                    if prev_av is not None:
                        prev_av()
                    prev_av = av
                prev_av()

            loop_cm = (tc.For_i(0, loop_k, 1) if loop_k
                       else contextlib.nullcontext())
            with loop_cm:
                # pipeline the two batches: b1's projections and b0's tail
                # are interleaved into the stream so the exp engine (the
                # bottleneck) never runs dry at boundaries
                st0, st1 = {}, {}
                proj_dma(0, st0)
                proj_a(0, st0)
                proj_b(0, st0)
                proj_c(0, st0)
                u0 = stream_units(0, st0)
                emit_stream(u0, {
                    8: [lambda: proj_dma(1, st1), lambda: proj_a(1, st1)],
                    14: [lambda: proj_b(1, st1)],
                    20: [lambda: proj_c(1, st1)],
                })
                tail_norm(0, st0)
                u1 = stream_units(1, st1)
                emit_stream(u1, {8: [lambda: tail_final(0, st0)]})
                tail_norm(1, st1)
                tail_final(1, st1)

    nc.compile()
    return nc


def _prep_weights(W_query, W_key, W_val, W1, W2, W3, W4, W_out):
    nf = 0.25  # 1/sqrt(16), exact power of two
    stack = lambda w: np.ascontiguousarray(
        np.asarray(w, np.float32).transpose(1, 0, 2).reshape(D, D))
    wq = stack(W_query) * nf
    wk = stack(W_key)
    mask = np.zeros((1, D), np.float32)
    for h in range(H):
        if h % 2 == 0:
            mask[0, h * KD:(h + 1) * KD] = 1.0
    wke = wk * mask
    wko = wk * (1.0 - mask)
    wo = np.asarray(W_out, np.float32)
    # W_out rows interleaved into 32-row groups: rows 32j+v hold head
    # (quad*4+j) vector v, rows 32j+16.. (denominator rows) are zero
    wo_pad = np.zeros((2, D, D), np.float32)
    for quad in range(2):
        for j in range(4):
            wo_pad[quad, 32 * j:32 * j + KD] = wo[quad * 4 + j]
    # sel[p, p'] = 1 iff p is the denominator row of p's 32-group
    sel = np.zeros((D, D), np.float32)
    for p2 in range(D):
        sel[32 * (p2 // 32) + 16, p2] = 1.0
    return {
        "wq": wq, "wke": wke, "wko": wko,
        "w1": stack(W1) * nf, "w2": stack(W2) * nf,
        "w3": stack(W3) * nf, "w4": stack(W4) * nf,
        "wv": stack(W_val), "sel": sel,
        "wo0": wo_pad[0].astype(ml_dtypes.bfloat16),
        "wo1": wo_pad[1].astype(ml_dtypes.bfloat16),
    }


def _numpy_fallback(q, W_query, W_key, W_val, W1, W2, W3, W4, W_out,
                    n_pick, n_delivery):
    """Pure-numpy reference for unexpected n_pick/n_delivery (not used for
    the standard 250/250 problem)."""
    h = np.asarray(q, np.float64)
    Bq, Gq, _ = h.shape
    nf = 1.0 / math.sqrt(KD)
    NEG = -np.inf
    proj = lambda x, W: np.einsum("bnd,hdk->hbnk", x, np.asarray(W, np.float64))
    sc = lambda Q, K: nf * np.einsum("hbqk,hbgk->hbqg", Q, K)
    zm = lambda c: np.where(c == 0, NEG, c)
    Q, K, V = proj(h, W_query), proj(h, W_key), proj(h, W_val)
    comp = sc(Q, K)
    hp, hd = h[:, 1:1 + n_pick], h[:, 1 + n_pick:]
    Kp, Vp = proj(hp, W_key), proj(hp, W_val)
    Kd, Vd = proj(hd, W_key), proj(hd, W_val)
    c_pp = zm(sc(proj(hp, W1), Kp))
    c_pd = zm(sc(proj(hp, W2), Kd))
    c_dp = zm(sc(proj(hd, W3), Kp))
    c_dd = zm(sc(proj(hd, W4), Kd))

    def place(blk, r0):
        full = np.full((H, Bq, Gq, blk.shape[3]), NEG)
        full[:, :, r0:r0 + blk.shape[2], :] = blk
        return full

    md = hd.shape[1]
    cf = np.concatenate([comp, place(c_pp, 1), place(c_pd, 1),
                         place(c_dd, Gq - md), place(c_dp, Gq - md)], axis=-1)
    cf -= cf.max(axis=-1, keepdims=True)
    e = np.exp(cf)
    attn = e / e.sum(axis=-1, keepdims=True)
    g, mp = Gq, n_pick
    heads = np.einsum("hbqg,hbgv->hbqv", attn[..., :g], V)
    heads += np.einsum("hbqp,hbpv->hbqv", attn[..., g:g + mp], Vp)
    heads += np.einsum("hbqd,hbdv->hbqv", attn[..., g + mp:g + mp + md], Vd)
    heads += np.einsum("hbqd,hbdv->hbqv",
                       attn[..., g + mp + md:g + mp + 2 * md], Vd)
    heads += np.einsum("hbqp,hbpv->hbqv", attn[..., g + mp + 2 * md:], Vp)
    return np.einsum("hbqv,hve->bqe", heads,
                     np.asarray(W_out, np.float64)).astype(np.float32)


def kernel(q, W_query, W_key, W_val, W1_query, W2_query, W3_query, W4_query,
           W_out, n_pick, n_delivery):
    np_, nd_ = int(n_pick), int(n_delivery)
    q = np.asarray(q, np.float32)
    if np_ != NP or nd_ != ND or q.shape != (B, G, D):
        return _numpy_fallback(q, W_query, W_key, W_val, W1_query, W2_query,
                               W3_query, W4_query, W_out, np_, nd_)

    from concourse import bass_utils

    if "nc" not in _CACHE:
        _CACHE["nc"] = _build_nc()
    nc = _CACHE["nc"]

    w = _prep_weights(W_query, W_key, W_val, W1_query, W2_query, W3_query,
                      W4_query, W_out)
    # host layout: rotate g axis (picks, delivs, depot) and transpose to
    # [b, d, g] so each core DMAs contiguous [128, 501] tiles
    hTr = np.ascontiguousarray(q[:, ROT, :].transpose(0, 2, 1))

    in_maps = [dict(w, hq=hTr[BPC * c:BPC * (c + 1)]) for c in range(NCORES)]
    res = bass_utils.run_bass_kernel_spmd(nc, in_maps,
                                          core_ids=list(range(NCORES)))
    return np.concatenate([r["out"] for r in res.results], axis=0)
